# revision 7
# baseline (speedup 1.0000x reference)
"""Trainium2 Bass kernel for nn_BlockGNN (2-layer message-passing GNN).

Strategy (8 NeuronCores, SPMD single program):
- Edges sharded by dst range: core c owns nodes [c*NPC, (c+1)*NPC) and all
  edges targeting them -> segment-sum is fully core-local.
- Per core, edges are packed (block-of-128-dst, src-chunk) run-padded so the
  instruction schedule is static and identical across cores.
- T-form pipeline (features on partitions):
    Z^T[h,e] = P[src]^T (+PE transpose of gathered rows)
             + Q_blk^T @ St  (dst one-hot expansion, sequential block reads)
             + W1c^T @ edge_attr^T (R-matmul)
    H^T = relu(Z^T + b1)         -> layer 0 stores H0^T for layer 1 reuse
    M[e,f] = (H^T_sub)^T @ W2 + b2
    agg^T += M_sub^T @ S_sub     (one-hot segment matmul)
- Layer 1 edge_attr transform is host-fused: V = W2_0 @ W1c_1 applied to H0^T.
- Node-update MLP + final MLP run data-parallel over the local node shard.
- Between layers only the src-side table P1 = nodes1 @ W1a_1 is AllGathered.
- Layer-0 src rows (P0[src]) are pre-gathered on host; layer-1 uses the
  dma_gather fast path (int16 idx, <=32k-row chunks, 2 SWDGE queues).
"""
import numpy as np

import concourse.bass as bass
import concourse.bacc as bacc
import concourse.mybir as mybir
import concourse.tile as tile
from concourse.bass_utils import run_bass_kernel_spmd
from concourse.library_config import mlp as _mlp_lib
from concourse.masks import make_identity

P = 128
F32 = mybir.dt.float32
I16 = mybir.dt.int16
AF = mybir.ActivationFunctionType
OP = mybir.AluOpType


class Cfg:
    def __init__(self, n, e, cores=8, chunks=4):
        self.n = n
        self.e = e
        self.cores = cores
        self.chunks = chunks
        self.npc = -(-n // (cores * P)) * P          # nodes per core, mult of 128
        self.n_pad = self.npc * cores
        self.blocks = self.npc // P                  # dst blocks per core
        self.chunk_rows = self.n_pad // chunks       # src chunk size
        assert self.chunk_rows % P == 0 and self.chunk_rows <= 32768


CFG_FULL = Cfg(n=100000, e=600000)


# ----------------------------------------------------------------------
# host-side packing
# ----------------------------------------------------------------------

def host_prepare(cfg, inputs):
    n, e = cfg.n, cfg.e
    src = np.asarray(inputs["edge_index"][0], dtype=np.int64).astype(np.int32)
    dst = np.asarray(inputs["edge_index"][1], dtype=np.int64).astype(np.int32)
    nodes = np.asarray(inputs["nodes"], dtype=np.float32)
    edge_attr = np.asarray(inputs["edge_attr"], dtype=np.float32)

    core = dst // cfg.npc
    blk = (dst % cfg.npc) // P
    chunk = src // cfg.chunk_rows

    # run = (core, chunk, blk); count fills
    run_id = (core * cfg.chunks + chunk) * cfg.blocks + blk
    n_runs = cfg.cores * cfg.chunks * cfg.blocks
    fills = np.bincount(run_id, minlength=n_runs)
    k_run = max(2, int(-(-fills.max() // P)))        # subtiles per run
    run_slots = k_run * P
    e_slots = cfg.chunks * cfg.blocks * run_slots    # per core

    # slot of each edge
    order = np.argsort(run_id, kind="stable")
    starts = np.zeros(n_runs + 1, np.int64)
    np.cumsum(fills, out=starts[1:])
    pos_in_run = np.empty(e, np.int64)
    pos_in_run[order] = np.arange(e) - starts[run_id[order]]
    core_local_run = (chunk * cfg.blocks + blk).astype(np.int64)
    slot = core_local_run * run_slots + pos_in_run   # within core

    # per-core arrays
    w = lambda key: np.asarray(inputs[key], dtype=np.float32)
    msg_w1, msg_b1 = w("msg_w1"), w("msg_b1")
    msg_w2, msg_b2 = w("msg_w2"), w("msg_b2")
    W1a = [msg_w1[l, 0:P, :] for l in range(2)]
    W1b = [msg_w1[l, P:2 * P, :] for l in range(2)]
    W1c = [msg_w1[l, 2 * P:3 * P, :] for l in range(2)]

    nodes_pad = np.zeros((cfg.n_pad, P), np.float32)
    nodes_pad[:n] = nodes
    P0 = nodes_pad @ W1a[0]                          # layer-0 src table

    V = msg_w2[0] @ W1c[1]
    c_row = msg_b2[0] @ W1c[1]
    b1_l0 = msg_b1[0][:, None].copy()
    b1_l1 = (msg_b1[1] + c_row)[:, None].copy()
    B2 = [np.tile(msg_b2[l][None, :], (P, 1)).copy() for l in range(2)]

    node_w1, node_b1 = w("node_w1"), w("node_b1")
    node_w2, node_b2 = w("node_w2"), w("node_b2")

    iota_tile = np.tile(np.arange(P, dtype=np.float32)[None, :], (P, 1)).copy()
    iota_col = np.arange(P, dtype=np.float32)[:, None].copy()

    shared = {
        "w1c0": W1c[0], "vmat": V, "w2_0": msg_w2[0], "w2_1": msg_w2[1],
        "b1_0": b1_l0, "b1_1": b1_l1, "b2t_0": B2[0], "b2t_1": B2[1],
        "w1a_1": W1a[1], "w1b_0": W1b[0], "w1b_1": W1b[1],
        "nw1a_0": node_w1[0, 0:P, :], "nw1b_0": node_w1[0, P:2 * P, :],
        "nw1a_1": node_w1[1, 0:P, :], "nw1b_1": node_w1[1, P:2 * P, :],
        "nw2_0": node_w2[0], "nw2_1": node_w2[1],
        "nb1_0": node_b1[0][:, None].copy(), "nb1_1": node_b1[1][:, None].copy(),
        "nb2_0": node_b2[0][:, None].copy(), "nb2_1": node_b2[1][:, None].copy(),
        "fw1": w("fin_w1"), "fw2": w("fin_w2"),
        "fb1": w("fin_b1")[:, None].copy(), "fb2": w("fin_b2")[:, None].copy(),
        "iota_t": iota_tile, "iota_c": iota_col,
    }
    shared = {k: np.ascontiguousarray(v, dtype=np.float32) for k, v in shared.items()}

    subt = e_slots // P
    n_batch = e_slots // (2 * run_slots) * 2         # runs gathered in pairs
    nb = cfg.chunks * cfg.blocks // 2                # gather batches (2 runs each)
    assert (cfg.chunks * cfg.blocks) % 2 == 0
    gb_idx = 2 * run_slots                           # idxs per gather batch

    in_maps = []
    slot2edge_all = []
    for c in range(cfg.cores):
        mask = core == c
        es = src[mask]
        ed = dst[mask]
        eslot = slot[mask]
        eid = np.nonzero(mask)[0]

        slot2edge = np.full(e_slots, -1, np.int64)
        slot2edge[eslot] = eid
        slot2edge_all.append(slot2edge)

        src_slot = np.zeros(e_slots, np.int32)
        src_slot[eslot] = es
        dstrel = np.full(e_slots, 200.0, np.float32)
        dstrel[eslot] = (ed % P).astype(np.float32)

        # layer-0 pre-gathered src rows, laid out [nb, 128, gb/128 * 128]
        ps0 = P0[src_slot]                           # [e_slots, 128]
        ps0 = ps0.reshape(nb, gb_idx // P, P, P).transpose(0, 2, 1, 3)
        ps0 = np.ascontiguousarray(ps0.reshape(nb, P, gb_idx)).astype(np.float32)

        # layer-1 gather idx (chunk-local, int16), wrapped-16 + replicated-8
        loc = (src_slot % cfg.chunk_rows).astype(np.int16)
        idx16 = np.zeros((nb, P, gb_idx // 16), np.int16)
        li = loc.reshape(nb, gb_idx)
        for rep in range(8):
            idx16[:, rep * 16:(rep + 1) * 16, :] = (
                li.reshape(nb, gb_idx // 16, 16).transpose(0, 2, 1)
            )

        eaT = np.zeros((e_slots, P), np.float32)
        eaT[eslot] = edge_attr[eid]
        eaT = np.ascontiguousarray(eaT.T)            # [128, e_slots]

        n0T = np.ascontiguousarray(nodes_pad[c * cfg.npc:(c + 1) * cfg.npc].T)

        im = dict(shared)
        im.update({
            "ps0": ps0,
            "idx16": idx16,
            "dstrel": dstrel.reshape(subt, P),
            "eaT": eaT,
            "n0T": n0T,
        })
        in_maps.append(im)

    meta = dict(k_run=k_run, e_slots=e_slots, nb=nb, gb_idx=gb_idx,
                slot2edge=slot2edge_all)
    return in_maps, meta


# ----------------------------------------------------------------------
# device kernel
# ----------------------------------------------------------------------

def build_nc(cfg, k_run, debug=False):
    run_slots = k_run * P
    e_slots = cfg.chunks * cfg.blocks * run_slots
    subt = e_slots // P
    nb = cfg.chunks * cfg.blocks // 2
    gb_idx = 2 * run_slots
    n_runs = cfg.chunks * cfg.blocks
    npc = cfg.npc

    nc = bacc.Bacc("TRN2", debug=False, num_swdge_queues=2)

    dp = lambda name, shape, dt=F32: nc.declare_dram_parameter(name, shape, dt, isOutput=False)
    d_ps0 = dp("ps0", [nb, P, gb_idx])
    d_idx16 = dp("idx16", [nb, P, gb_idx // 16], I16)
    d_dstrel = dp("dstrel", [subt, P])
    d_eaT = dp("eaT", [P, e_slots])
    d_n0T = dp("n0T", [P, npc])
    wnames = ["w1c0", "vmat", "w2_0", "w2_1", "b2t_0", "b2t_1", "w1a_1",
              "w1b_0", "w1b_1", "nw1a_0", "nw1b_0", "nw1a_1", "nw1b_1",
              "nw2_0", "nw2_1", "fw1", "fw2", "iota_t"]
    dW = {k: dp(k, [P, P]) for k in wnames}
    cnames = ["b1_0", "b1_1", "nb1_0", "nb1_1", "nb2_0", "nb2_1", "fb1",
              "fb2", "iota_c"]
    dC = {k: dp(k, [P, 1]) for k in cnames}

    d_outT = nc.declare_dram_parameter("out_nodesT", [P, npc], F32, isOutput=True)
    d_eout = nc.declare_dram_parameter("edge_out", [e_slots, P], F32, isOutput=True)
    if debug:
        d_dbg = {
            "dbg_q0": nc.declare_dram_parameter("dbg_q0", [P, cfg.blocks * P], F32, isOutput=True),
            "dbg_h0": nc.declare_dram_parameter("dbg_h0", [P, e_slots], F32, isOutput=True),
            "dbg_agg0": nc.declare_dram_parameter("dbg_agg0", [P, npc], F32, isOutput=True),
            "dbg_n1": nc.declare_dram_parameter("dbg_n1", [P, npc], F32, isOutput=True),
            "dbg_p1f": nc.declare_dram_parameter("dbg_p1f", [cfg.n_pad, P], F32, isOutput=True),
            "dbg_g1": nc.declare_dram_parameter("dbg_g1", [P, gb_idx], F32, isOutput=True),
        }

    with tile.TileContext(nc) as tc:
        with (
            tc.tile_pool(name="const", bufs=1) as cp,
            tc.tile_pool(name="tabs", bufs=1) as tp,
            tc.tile_pool(name="dram", bufs=1, space="DRAM") as dr,
            tc.tile_pool(name="stream", bufs=3) as sp,
            tc.tile_pool(name="psum", bufs=2, space="PSUM") as pp,
        ):
            nc.gpsimd.load_library(_mlp_lib)
            ident = cp.tile([P, P], F32)
            make_identity(nc, ident[:])
            W = {}
            for k in wnames:
                W[k] = cp.tile([P, P], F32, name=f"w_{k}", tag=f"w_{k}")
                nc.sync.dma_start(W[k][:], dW[k][:])
            Cw = {}
            for k in cnames:
                Cw[k] = cp.tile([P, 1], F32, name=f"c_{k}", tag=f"c_{k}")
                nc.sync.dma_start(Cw[k][:], dC[k][:])

            # internal DRAM
            d_H0T = dr.tile([P, e_slots], F32)
            d_n1T = dr.tile([P, npc], F32)
            d_p1b = dr.tile([npc, P], F32)
            d_p1f = dr.tile([cfg.n_pad, P], F32)

            def q_table_prep(q_tile, d_srcT, w1b_key):
                """Q[nodes,h] per block from a nodesT shard in DRAM."""
                for g in range(0, cfg.blocks, 4):
                    gw = min(4, cfg.blocks - g)
                    stg = sp.tile([P, 4 * P], F32, tag="qstage")
                    nc.sync.dma_start(stg[:, :gw * P],
                                      d_srcT[:, g * P:(g + gw) * P])
                    for k in range(gw):
                        ps = pp.tile([P, P], F32, tag="agg")
                        nc.tensor.matmul(ps[:], lhsT=stg[:, k * P:(k + 1) * P],
                                         rhs=W[w1b_key][:], start=True, stop=True)
                        b = g + k
                        nc.vector.tensor_copy(q_tile[:, b * P:(b + 1) * P], ps[:])

            def edge_phase(layer):
                q_tile = tp.tile([P, cfg.blocks * P], F32, tag="qtab")
                agg = tp.tile([P, npc], F32, tag="agg_sb")
                if layer == 0:
                    q_table_prep(q_tile, d_n0T, "w1b_0")
                else:
                    q_table_prep(q_tile, d_n1T, "w1b_1")
                nc.vector.memset(agg[:], 0.0)

                w_r = W["w1c0"] if layer == 0 else W["vmat"]
                w_2 = W["w2_0"] if layer == 0 else W["w2_1"]
                b2t = W["b2t_0"] if layer == 0 else W["b2t_1"]
                b1 = Cw["b1_0"] if layer == 0 else Cw["b1_1"]

                for batch in range(nb):
                    gtile = sp.tile([P, gb_idx], F32, tag="G")
                    if layer == 0:
                        nc.sync.dma_start(gtile[:], d_ps0[batch])
                    else:
                        it = sp.tile([P, gb_idx // 16], I16, tag="idx")
                        nc.gpsimd.dma_start(it[:], d_idx16[batch])
                        chunk = (batch * 2) // cfg.blocks
                        cr = cfg.chunk_rows
                        nc.gpsimd.dma_gather(
                            gtile[:].rearrange("p (t f) -> p t f", f=P),
                            d_p1f[chunk * cr:(chunk + 1) * cr, :],
                            it[:], gb_idx, gb_idx, P,
                            queue_num=batch % 2,
                        )
                    for half in range(2):
                        r = batch * 2 + half          # run index
                        blkb = r % cfg.blocks         # dst block
                        s0 = r * k_run                # first subtile
                        zp = pp.tile([P, run_slots], F32, tag="z")
                        drow = d_dstrel[s0:s0 + k_run, :]
                        dst_b = sp.tile([P, run_slots], F32, tag="dstb")
                        nc.sync.dma_start(
                            dst_b[:],
                            drow.rearrange("s p -> (s p)")[None, :]
                                .to_broadcast([P, run_slots]))
                        dst_c = sp.tile([P, k_run], F32, tag="dstc")
                        nc.sync.dma_start(dst_c[:], drow.rearrange("s p -> p s"))
                        st_t = sp.tile([P, run_slots], F32, tag="St")
                        nc.vector.tensor_scalar(
                            out=st_t[:], in0=dst_b[:], scalar1=Cw["iota_c"][:, :1],
                            scalar2=None, op0=OP.is_equal)
                        s_t = sp.tile([P, run_slots], F32, tag="S")
                        for j in range(k_run):
                            jr = slice(j * P, (j + 1) * P)
                            nc.vector.tensor_scalar(
                                out=s_t[:, jr], in0=W["iota_t"][:],
                                scalar1=dst_c[:, j:j + 1], scalar2=None,
                                op0=OP.is_equal)
                        rt = sp.tile([P, run_slots], F32, tag="rt")
                        e0 = r * run_slots
                        if layer == 0:
                            nc.sync.dma_start(rt[:], d_eaT[:, e0:e0 + run_slots])
                        else:
                            nc.sync.dma_start(rt[:], d_H0T[:, e0:e0 + run_slots])
                        nc.tensor.matmul(out=zp[:], lhsT=w_r[:], rhs=rt[:],
                                         start=True, stop=False,
                                         skip_group_check=True)
                        hoff = half * run_slots
                        for j in range(k_run):
                            jr = slice(j * P, (j + 1) * P)
                            gj = slice(hoff + j * P, hoff + (j + 1) * P)
                            nc.tensor.matmul(
                                out=zp[:, jr], lhsT=gtile[:, gj], rhs=ident[:],
                                is_transpose=True, start=False, stop=False,
                                skip_group_check=True)
                            nc.tensor.matmul(
                                out=zp[:, jr],
                                lhsT=q_tile[:, blkb * P:(blkb + 1) * P],
                                rhs=st_t[:, jr], start=False,
                                stop=(j == k_run - 1),
                                skip_group_check=True)
                        ht = sp.tile([P, run_slots], F32, tag="ht")
                        nc.scalar.activation(out=ht[:], in_=zp[:], func=AF.Relu,
                                             bias=b1[:, :1])
                        if layer == 0:
                            nc.sync.dma_start(d_H0T[:, e0:e0 + run_slots], ht[:])
                        mp = pp.tile([P, run_slots], F32, tag="m")
                        for j in range(k_run):
                            jr = slice(j * P, (j + 1) * P)
                            nc.tensor.matmul(out=mp[:, jr], lhsT=ht[:, jr],
                                             rhs=w_2[:], start=True, stop=True)
                        mt = sp.tile([P, run_slots], F32, tag="mt")
                        for j in range(k_run):
                            jr = slice(j * P, (j + 1) * P)
                            nc.vector.tensor_tensor(out=mt[:, jr], in0=mp[:, jr],
                                                    in1=b2t[:], op=OP.add)
                        ap_ = pp.tile([P, P], F32, tag="agg")
                        for j in range(k_run):
                            jr = slice(j * P, (j + 1) * P)
                            nc.tensor.matmul(out=ap_[:], lhsT=mt[:, jr],
                                             rhs=s_t[:, jr], start=(j == 0),
                                             stop=(j == k_run - 1))
                        bcol = slice(blkb * P, (blkb + 1) * P)
                        nc.vector.tensor_tensor(out=agg[:, bcol],
                                                in0=agg[:, bcol], in1=ap_[:],
                                                op=OP.add)
                        if layer == 1:
                            nc.sync.dma_start(
                                d_eout[e0:e0 + run_slots, :]
                                    .rearrange("(j p) f -> p j f", p=P),
                                mt[:].rearrange("p (j f) -> p j f", f=P))
                if debug and layer == 0:
                    nc.sync.dma_start(d_dbg["dbg_q0"][:], q_tile[:])
                    nc.sync.dma_start(d_dbg["dbg_agg0"][:], agg[:])
                return agg

            def node_phase(layer, agg):
                """Node MLP over the local shard; returns nothing (DMAs out)."""
                src_dram = d_n0T if layer == 0 else d_n1T
                nw1a = W["nw1a_0"] if layer == 0 else W["nw1a_1"]
                nw1b = W["nw1b_0"] if layer == 0 else W["nw1b_1"]
                nw2 = W["nw2_0"] if layer == 0 else W["nw2_1"]
                nb1 = Cw["nb1_0"] if layer == 0 else Cw["nb1_1"]
                nb2 = Cw["nb2_0"] if layer == 0 else Cw["nb2_1"]
                for g0 in range(0, npc, 512):
                    gw = min(512, npc - g0)
                    nt = sp.tile([P, 512], F32, tag="nt")
                    nc.sync.dma_start(nt[:, :gw], src_dram[:, g0:g0 + gw])
                    hp = pp.tile([P, 512], F32, tag="z")
                    nc.tensor.matmul(out=hp[:, :gw], lhsT=nw1a[:],
                                     rhs=nt[:, :gw], start=True, stop=False,
                                     skip_group_check=True)
                    nc.tensor.matmul(out=hp[:, :gw], lhsT=nw1b[:],
                                     rhs=agg[:, g0:g0 + gw], start=False,
                                     stop=True, skip_group_check=True)
                    hn = sp.tile([P, 512], F32, tag="hn")
                    nc.scalar.activation(out=hn[:, :gw], in_=hp[:, :gw],
                                         func=AF.Relu, bias=nb1[:, :1])
                    op_ = pp.tile([P, 512], F32, tag="m")
                    nc.tensor.matmul(out=op_[:, :gw], lhsT=nw2[:], rhs=hn[:, :gw],
                                     start=True, stop=True)
                    o_t = sp.tile([P, 512], F32, tag="ot")
                    nc.scalar.activation(out=o_t[:, :gw], in_=op_[:, :gw],
                                         func=AF.Identity, bias=nb2[:, :1])
                    if layer == 0:
                        nc.sync.dma_start(d_n1T[:, g0:g0 + gw], o_t[:, :gw])
                        # P1 shard rows + feed AllGather later
                        for k in range(0, gw, P):
                            pps = pp.tile([P, P], F32, tag="agg")
                            nc.tensor.matmul(pps[:], lhsT=o_t[:, k:k + P],
                                             rhs=W["w1a_1"][:], start=True,
                                             stop=True)
                            stg = sp.tile([P, P], F32, tag="p1stage")
                            nc.vector.tensor_copy(stg[:], pps[:])
                            nc.sync.dma_start(d_p1b[g0 + k:g0 + k + P, :], stg[:])
                    else:
                        fp = pp.tile([P, 512], F32, tag="z")
                        nc.tensor.matmul(out=fp[:, :gw], lhsT=W["fw1"][:],
                                         rhs=o_t[:, :gw], start=True, stop=True)
                        fh = sp.tile([P, 512], F32, tag="fh")
                        nc.scalar.activation(out=fh[:, :gw], in_=fp[:, :gw],
                                             func=AF.Relu, bias=Cw["fb1"][:, :1])
                        fo = pp.tile([P, 512], F32, tag="m")
                        nc.tensor.matmul(out=fo[:, :gw], lhsT=W["fw2"][:],
                                         rhs=fh[:, :gw], start=True, stop=True)
                        fs = sp.tile([P, 512], F32, tag="fs")
                        nc.scalar.activation(out=fs[:, :gw], in_=fo[:, :gw],
                                             func=AF.Identity, bias=Cw["fb2"][:, :1])
                        nc.sync.dma_start(d_outT[:, g0:g0 + gw], fs[:, :gw])

            agg0 = edge_phase(0)
            node_phase(0, agg0)
            if debug:
                st = sp.tile([P, 512], F32, tag="dbgst")
                for g0 in range(0, e_slots, 512):
                    gw = min(512, e_slots - g0)
                    nc.sync.dma_start(st[:, :gw], d_H0T[:, g0:g0 + gw])
                    nc.sync.dma_start(d_dbg["dbg_h0"][:, g0:g0 + gw], st[:, :gw])
                for g0 in range(0, npc, 512):
                    gw = min(512, npc - g0)
                    nc.sync.dma_start(st[:, :gw], d_n1T[:, g0:g0 + gw])
                    nc.sync.dma_start(d_dbg["dbg_n1"][:, g0:g0 + gw], st[:, :gw])
            nc.gpsimd.collective_compute(
                "AllGather", OP.bypass,
                replica_groups=[list(range(cfg.cores))],
                ins=[d_p1b[:].opt()], outs=[d_p1f[:].opt()])
            if debug:
                st2 = sp.tile([P, 512], F32, tag="dbgst2")
                for g0 in range(0, cfg.n_pad, 512):
                    gw = min(512, cfg.n_pad - g0)
                    nc.sync.dma_start(st2[:, :gw],
                                      d_p1f[g0:g0 + gw, :].rearrange("(a p) f -> p (a f)", p=P)
                                      if False else d_p1f[g0:g0 + gw, :].rearrange("n f -> f n"))
                    nc.sync.dma_start(d_dbg["dbg_p1f"][g0:g0 + gw, :].rearrange("n f -> f n"), st2[:, :gw])
            agg1 = edge_phase(1)
            node_phase(1, agg1)

    nc.compile()
    return nc


# ----------------------------------------------------------------------
# top level
# ----------------------------------------------------------------------

def _assemble(cfg, meta, results):
    n_pad, npc = cfg.n_pad, cfg.npc
    nodesT = np.concatenate([results[c]["out_nodesT"] for c in range(cfg.cores)],
                            axis=1)
    nodes_out = np.ascontiguousarray(nodesT.T[:cfg.n]).astype(np.float32)
    edge_out = np.empty((cfg.e, P), np.float32)
    for c in range(cfg.cores):
        s2e = meta["slot2edge"][c]
        m = s2e >= 0
        edge_out[s2e[m]] = results[c]["edge_out"][m]
    return nodes_out, edge_out


def run_cfg(cfg, inputs, runner=None):
    in_maps, meta = host_prepare(cfg, inputs)
    nc = build_nc(cfg, meta["k_run"])
    res = run_bass_kernel_spmd(nc, in_maps, core_ids=list(range(cfg.cores)))
    return _assemble(cfg, meta, res.results)


def kernel(**inputs):
    return run_cfg(CFG_FULL, inputs)


# revision 10
# speedup vs baseline: 86.3494x; 86.3494x over previous
"""Trainium2 Bass kernel for nn_BlockGNN (2-layer message-passing GNN).

Strategy (8 NeuronCores, SPMD single program):
- Edges sharded by dst range: core c owns nodes [c*NPC, (c+1)*NPC) and all
  edges targeting them -> segment-sum is fully core-local.
- Per core, edges are packed (block-of-128-dst, src-chunk) run-padded so the
  instruction schedule is static and identical across cores.
- T-form pipeline (features on partitions):
    Z^T[h,e] = P[src]^T (+PE transpose of gathered rows)
             + Q_blk^T @ St  (dst one-hot expansion, sequential block reads)
             + W1c^T @ edge_attr^T (R-matmul)
    H^T = relu(Z^T + b1)         -> layer 0 stores H0^T for layer 1 reuse
    M[e,f] = (H^T_sub)^T @ W2 + b2
    agg^T += M_sub^T @ S_sub     (one-hot segment matmul)
- Layer 1 edge_attr transform is host-fused: V = W2_0 @ W1c_1 applied to H0^T.
- Node-update MLP + final MLP run data-parallel over the local node shard.
- Between layers only the src-side table P1 = nodes1 @ W1a_1 is AllGathered.
- Layer-0 src rows (P0[src]) are pre-gathered on host; layer-1 uses the
  dma_gather fast path (int16 idx, <=32k-row chunks, 2 SWDGE queues).
"""
import numpy as np

import concourse.bass as bass
import concourse.bacc as bacc
import concourse.mybir as mybir
import concourse.tile as tile
from concourse.bass_utils import run_bass_kernel_spmd
from concourse.library_config import mlp as _mlp_lib
from concourse.masks import make_identity

P = 128
F32 = mybir.dt.float32
I16 = mybir.dt.int16
AF = mybir.ActivationFunctionType
OP = mybir.AluOpType


class Cfg:
    def __init__(self, n, e, cores=8, chunks=4):
        self.n = n
        self.e = e
        self.cores = cores
        self.chunks = chunks
        self.npc = -(-n // (cores * P)) * P          # nodes per core, mult of 128
        self.n_pad = self.npc * cores
        self.blocks = self.npc // P                  # dst blocks per core
        self.chunk_rows = self.n_pad // chunks       # src chunk size
        assert self.chunk_rows % P == 0 and self.chunk_rows <= 32768


CFG_FULL = Cfg(n=100000, e=600000)


# ----------------------------------------------------------------------
# host-side packing
# ----------------------------------------------------------------------

def host_prepare(cfg, inputs):
    n, e = cfg.n, cfg.e
    src = np.asarray(inputs["edge_index"][0], dtype=np.int64).astype(np.int32)
    dst = np.asarray(inputs["edge_index"][1], dtype=np.int64).astype(np.int32)
    nodes = np.asarray(inputs["nodes"], dtype=np.float32)
    edge_attr = np.asarray(inputs["edge_attr"], dtype=np.float32)

    core = dst // cfg.npc
    blk = (dst % cfg.npc) // P
    chunk = src // cfg.chunk_rows

    # run = (core, chunk, blk); count fills
    run_id = (core * cfg.chunks + chunk) * cfg.blocks + blk
    n_runs = cfg.cores * cfg.chunks * cfg.blocks
    fills = np.bincount(run_id, minlength=n_runs)
    k_run = max(2, int(-(-fills.max() // P)))        # subtiles per run
    run_slots = k_run * P
    e_slots = cfg.chunks * cfg.blocks * run_slots    # per core

    # slot of each edge
    order = np.argsort(run_id, kind="stable")
    starts = np.zeros(n_runs + 1, np.int64)
    np.cumsum(fills, out=starts[1:])
    pos_in_run = np.empty(e, np.int64)
    pos_in_run[order] = np.arange(e) - starts[run_id[order]]
    core_local_run = (chunk * cfg.blocks + blk).astype(np.int64)
    slot = core_local_run * run_slots + pos_in_run   # within core

    # per-core arrays
    w = lambda key: np.asarray(inputs[key], dtype=np.float32)
    msg_w1, msg_b1 = w("msg_w1"), w("msg_b1")
    msg_w2, msg_b2 = w("msg_w2"), w("msg_b2")
    W1a = [msg_w1[l, 0:P, :] for l in range(2)]
    W1b = [msg_w1[l, P:2 * P, :] for l in range(2)]
    W1c = [msg_w1[l, 2 * P:3 * P, :] for l in range(2)]

    nodes_pad = np.zeros((cfg.n_pad, P), np.float32)
    nodes_pad[:n] = nodes
    P0 = nodes_pad @ W1a[0]                          # layer-0 src table

    V = msg_w2[0] @ W1c[1]
    c_row = msg_b2[0] @ W1c[1]
    b1_l0 = msg_b1[0][:, None].copy()
    b1_l1 = (msg_b1[1] + c_row)[:, None].copy()
    B2 = [np.tile(msg_b2[l][None, :], (P, 1)).copy() for l in range(2)]

    node_w1, node_b1 = w("node_w1"), w("node_b1")
    node_w2, node_b2 = w("node_w2"), w("node_b2")

    iota_tile = np.tile(np.arange(P, dtype=np.float32)[None, :], (P, 1)).copy()
    iota_col = np.arange(P, dtype=np.float32)[:, None].copy()

    shared = {
        "w1c0": W1c[0], "vmat": V, "w2_0": msg_w2[0], "w2_1": msg_w2[1],
        "b1_0": b1_l0, "b1_1": b1_l1, "b2t_0": B2[0], "b2t_1": B2[1],
        "w1a_1": W1a[1], "w1b_0": W1b[0], "w1b_1": W1b[1],
        "nw1a_0": node_w1[0, 0:P, :], "nw1b_0": node_w1[0, P:2 * P, :],
        "nw1a_1": node_w1[1, 0:P, :], "nw1b_1": node_w1[1, P:2 * P, :],
        "nw2_0": node_w2[0], "nw2_1": node_w2[1],
        "nb1_0": node_b1[0][:, None].copy(), "nb1_1": node_b1[1][:, None].copy(),
        "nb2_0": node_b2[0][:, None].copy(), "nb2_1": node_b2[1][:, None].copy(),
        "fw1": w("fin_w1"), "fw2": w("fin_w2"),
        "fb1": w("fin_b1")[:, None].copy(), "fb2": w("fin_b2")[:, None].copy(),
        "iota_t": iota_tile, "iota_c": iota_col,
    }
    shared = {k: np.ascontiguousarray(v, dtype=np.float32) for k, v in shared.items()}

    subt = e_slots // P
    n_batch = e_slots // (2 * run_slots) * 2         # runs gathered in pairs
    nb = cfg.chunks * cfg.blocks // 2                # gather batches (2 runs each)
    assert (cfg.chunks * cfg.blocks) % 2 == 0
    gb_idx = 2 * run_slots                           # idxs per gather batch

    in_maps = []
    slot2edge_all = []
    for c in range(cfg.cores):
        mask = core == c
        es = src[mask]
        ed = dst[mask]
        eslot = slot[mask]
        eid = np.nonzero(mask)[0]

        slot2edge = np.full(e_slots, -1, np.int64)
        slot2edge[eslot] = eid
        slot2edge_all.append(slot2edge)

        src_slot = np.zeros(e_slots, np.int32)
        src_slot[eslot] = es
        dstrel = np.full(e_slots, 200.0, np.float32)
        dstrel[eslot] = (ed % P).astype(np.float32)

        # layer-0 pre-gathered src rows, laid out [nb, 128, gb/128 * 128]
        ps0 = P0[src_slot]                           # [e_slots, 128]
        ps0 = ps0.reshape(nb, gb_idx // P, P, P).transpose(0, 2, 1, 3)
        ps0 = np.ascontiguousarray(ps0.reshape(nb, P, gb_idx)).astype(np.float32)

        # layer-1 gather idx (chunk-local, int16), wrapped-16 + replicated-8
        loc = (src_slot % cfg.chunk_rows).astype(np.int16)
        idx16 = np.zeros((nb, P, gb_idx // 16), np.int16)
        li = loc.reshape(nb, gb_idx)
        for rep in range(8):
            idx16[:, rep * 16:(rep + 1) * 16, :] = (
                li.reshape(nb, gb_idx // 16, 16).transpose(0, 2, 1)
            )

        eaT = np.zeros((e_slots, P), np.float32)
        eaT[eslot] = edge_attr[eid]
        eaT = np.ascontiguousarray(eaT.T)            # [128, e_slots]

        n0T = np.ascontiguousarray(nodes_pad[c * cfg.npc:(c + 1) * cfg.npc].T)

        im = dict(shared)
        im.update({
            "ps0": ps0,
            "idx16": idx16,
            "dstrel": dstrel.reshape(subt, P),
            "eaT": eaT,
            "n0T": n0T,
        })
        in_maps.append(im)

    meta = dict(k_run=k_run, e_slots=e_slots, nb=nb, gb_idx=gb_idx,
                slot2edge=slot2edge_all)
    return in_maps, meta


# ----------------------------------------------------------------------
# device kernel
# ----------------------------------------------------------------------

def build_nc(cfg, k_run, debug=False, variant="full"):
    run_slots = k_run * P
    e_slots = cfg.chunks * cfg.blocks * run_slots
    subt = e_slots // P
    nb = cfg.chunks * cfg.blocks // 2
    gb_idx = 2 * run_slots
    n_runs = cfg.chunks * cfg.blocks
    npc = cfg.npc

    nc = bacc.Bacc("TRN2", debug=False, num_swdge_queues=2)

    dp = lambda name, shape, dt=F32: nc.declare_dram_parameter(name, shape, dt, isOutput=False)
    d_ps0 = dp("ps0", [nb, P, gb_idx])
    d_idx16 = dp("idx16", [nb, P, gb_idx // 16], I16)
    d_dstrel = dp("dstrel", [subt, P])
    d_eaT = dp("eaT", [P, e_slots])
    d_n0T = dp("n0T", [P, npc])
    wnames = ["w1c0", "vmat", "w2_0", "w2_1", "b2t_0", "b2t_1", "w1a_1",
              "w1b_0", "w1b_1", "nw1a_0", "nw1b_0", "nw1a_1", "nw1b_1",
              "nw2_0", "nw2_1", "fw1", "fw2", "iota_t"]
    dW = {k: dp(k, [P, P]) for k in wnames}
    cnames = ["b1_0", "b1_1", "nb1_0", "nb1_1", "nb2_0", "nb2_1", "fb1",
              "fb2", "iota_c"]
    dC = {k: dp(k, [P, 1]) for k in cnames}

    d_outT = nc.declare_dram_parameter("out_nodesT", [P, npc], F32, isOutput=True)
    d_eout = nc.declare_dram_parameter("edge_out", [e_slots, P], F32, isOutput=True)
    if debug:
        d_dbg = {
            "dbg_q0": nc.declare_dram_parameter("dbg_q0", [P, cfg.blocks * P], F32, isOutput=True),
            "dbg_h0": nc.declare_dram_parameter("dbg_h0", [P, e_slots], F32, isOutput=True),
            "dbg_agg0": nc.declare_dram_parameter("dbg_agg0", [P, npc], F32, isOutput=True),
            "dbg_n1": nc.declare_dram_parameter("dbg_n1", [P, npc], F32, isOutput=True),
            "dbg_p1f": nc.declare_dram_parameter("dbg_p1f", [cfg.n_pad, P], F32, isOutput=True),
            "dbg_g1": nc.declare_dram_parameter("dbg_g1", [P, gb_idx], F32, isOutput=True),
        }

    with tile.TileContext(nc) as tc:
        with (
            tc.tile_pool(name="const", bufs=1) as cp,
            tc.tile_pool(name="tabs", bufs=1) as tp,
            tc.tile_pool(name="dram", bufs=1, space="DRAM") as dr,
            tc.tile_pool(name="stream", bufs=3) as sp,
            tc.tile_pool(name="psum", bufs=2, space="PSUM") as pp,
        ):
            nc.gpsimd.load_library(_mlp_lib)
            ident = cp.tile([P, P], F32)
            make_identity(nc, ident[:])
            W = {}
            for k in wnames:
                W[k] = cp.tile([P, P], F32, name=f"w_{k}", tag=f"w_{k}")
                nc.sync.dma_start(W[k][:], dW[k][:])
            Cw = {}
            for k in cnames:
                Cw[k] = cp.tile([P, 1], F32, name=f"c_{k}", tag=f"c_{k}")
                nc.sync.dma_start(Cw[k][:], dC[k][:])

            # internal DRAM
            d_H0T = dr.tile([P, e_slots], F32)
            d_n1T = dr.tile([P, npc], F32)
            d_p1b = dr.tile([npc, P], F32)
            d_p1f = dr.tile([cfg.n_pad, P], F32, addr_space="Shared")

            def q_table_prep(q_tile, d_srcT, w1b_key):
                """Q[nodes,h] per block from a nodesT shard in DRAM."""
                for g in range(0, cfg.blocks, 4):
                    gw = min(4, cfg.blocks - g)
                    stg = sp.tile([P, 4 * P], F32, tag="qstage")
                    nc.sync.dma_start(stg[:, :gw * P],
                                      d_srcT[:, g * P:(g + gw) * P])
                    for k in range(gw):
                        ps = pp.tile([P, P], F32, tag="agg")
                        nc.tensor.matmul(ps[:], lhsT=stg[:, k * P:(k + 1) * P],
                                         rhs=W[w1b_key][:], start=True, stop=True)
                        b = g + k
                        nc.vector.tensor_copy(q_tile[:, b * P:(b + 1) * P], ps[:])

            def edge_phase(layer):
                q_tile = tp.tile([P, cfg.blocks * P], F32, tag="qtab")
                agg = tp.tile([P, npc], F32, tag="agg_sb")
                if layer == 0:
                    q_table_prep(q_tile, d_n0T, "w1b_0")
                else:
                    q_table_prep(q_tile, d_n1T, "w1b_1")
                nc.vector.memset(agg[:], 0.0)

                w_r = W["w1c0"] if layer == 0 else W["vmat"]
                w_2 = W["w2_0"] if layer == 0 else W["w2_1"]
                b2t = W["b2t_0"] if layer == 0 else W["b2t_1"]
                b1 = Cw["b1_0"] if layer == 0 else Cw["b1_1"]

                for batch in range(nb):
                    gtile = sp.tile([P, gb_idx], F32, tag="G")
                    if layer == 0:
                        nc.sync.dma_start(gtile[:], d_ps0[batch])
                    else:
                        it = sp.tile([P, gb_idx // 16], I16, tag="idx")
                        nc.gpsimd.dma_start(it[:], d_idx16[batch])
                        chunk = (batch * 2) // cfg.blocks
                        cr = cfg.chunk_rows
                        nc.gpsimd.dma_gather(
                            gtile[:].rearrange("p (t f) -> p t f", f=P),
                            d_p1f[chunk * cr:(chunk + 1) * cr, :],
                            it[:], gb_idx, gb_idx, P,
                            queue_num=batch % 2,
                        )
                    for half in range(2):
                        r = batch * 2 + half          # run index
                        blkb = r % cfg.blocks         # dst block
                        s0 = r * k_run                # first subtile
                        zp = pp.tile([P, run_slots], F32, tag="z")
                        drow = d_dstrel[s0:s0 + k_run, :]
                        dst_b = sp.tile([P, run_slots], F32, tag="dstb")
                        nc.sync.dma_start(
                            dst_b[:],
                            drow.rearrange("s p -> (s p)")[None, :]
                                .to_broadcast([P, run_slots]))
                        dst_c = sp.tile([P, k_run], F32, tag="dstc")
                        nc.sync.dma_start(dst_c[:], drow.rearrange("s p -> p s"))
                        st_t = sp.tile([P, run_slots], F32, tag="St")
                        nc.vector.tensor_scalar(
                            out=st_t[:], in0=dst_b[:], scalar1=Cw["iota_c"][:, :1],
                            scalar2=None, op0=OP.is_equal)
                        s_t = sp.tile([P, run_slots], F32, tag="S")
                        for j in range(k_run):
                            jr = slice(j * P, (j + 1) * P)
                            nc.vector.tensor_scalar(
                                out=s_t[:, jr], in0=W["iota_t"][:],
                                scalar1=dst_c[:, j:j + 1], scalar2=None,
                                op0=OP.is_equal)
                        rt = sp.tile([P, run_slots], F32, tag="rt")
                        e0 = r * run_slots
                        if layer == 0:
                            nc.sync.dma_start(rt[:], d_eaT[:, e0:e0 + run_slots])
                        else:
                            nc.sync.dma_start(rt[:], d_H0T[:, e0:e0 + run_slots])
                        nc.tensor.matmul(out=zp[:], lhsT=w_r[:], rhs=rt[:],
                                         start=True, stop=False,
                                         skip_group_check=True)
                        hoff = half * run_slots
                        for j in range(k_run):
                            jr = slice(j * P, (j + 1) * P)
                            gj = slice(hoff + j * P, hoff + (j + 1) * P)
                            nc.tensor.matmul(
                                out=zp[:, jr], lhsT=gtile[:, gj], rhs=ident[:],
                                is_transpose=True, start=False, stop=False,
                                skip_group_check=True)
                            nc.tensor.matmul(
                                out=zp[:, jr],
                                lhsT=q_tile[:, blkb * P:(blkb + 1) * P],
                                rhs=st_t[:, jr], start=False,
                                stop=(j == k_run - 1),
                                skip_group_check=True)
                        ht = sp.tile([P, run_slots], F32, tag="ht")
                        nc.scalar.activation(out=ht[:], in_=zp[:], func=AF.Relu,
                                             bias=b1[:, :1])
                        if layer == 0:
                            nc.sync.dma_start(d_H0T[:, e0:e0 + run_slots], ht[:])
                        mp = pp.tile([P, run_slots], F32, tag="m")
                        for j in range(k_run):
                            jr = slice(j * P, (j + 1) * P)
                            nc.tensor.matmul(out=mp[:, jr], lhsT=ht[:, jr],
                                             rhs=w_2[:], start=True, stop=True)
                        mt = sp.tile([P, run_slots], F32, tag="mt")
                        for j in range(k_run):
                            jr = slice(j * P, (j + 1) * P)
                            nc.vector.tensor_tensor(out=mt[:, jr], in0=mp[:, jr],
                                                    in1=b2t[:], op=OP.add)
                        ap_ = pp.tile([P, P], F32, tag="agg")
                        for j in range(k_run):
                            jr = slice(j * P, (j + 1) * P)
                            nc.tensor.matmul(out=ap_[:], lhsT=mt[:, jr],
                                             rhs=s_t[:, jr], start=(j == 0),
                                             stop=(j == k_run - 1))
                        bcol = slice(blkb * P, (blkb + 1) * P)
                        nc.vector.tensor_tensor(out=agg[:, bcol],
                                                in0=agg[:, bcol], in1=ap_[:],
                                                op=OP.add)
                        if layer == 1:
                            nc.sync.dma_start(
                                d_eout[e0:e0 + run_slots, :]
                                    .rearrange("(j p) f -> p j f", p=P),
                                mt[:].rearrange("p (j f) -> p j f", f=P))
                if debug and layer == 0:
                    nc.sync.dma_start(d_dbg["dbg_q0"][:], q_tile[:])
                    nc.sync.dma_start(d_dbg["dbg_agg0"][:], agg[:])
                return agg

            def node_phase(layer, agg):
                """Node MLP over the local shard; returns nothing (DMAs out)."""
                src_dram = d_n0T if layer == 0 else d_n1T
                nw1a = W["nw1a_0"] if layer == 0 else W["nw1a_1"]
                nw1b = W["nw1b_0"] if layer == 0 else W["nw1b_1"]
                nw2 = W["nw2_0"] if layer == 0 else W["nw2_1"]
                nb1 = Cw["nb1_0"] if layer == 0 else Cw["nb1_1"]
                nb2 = Cw["nb2_0"] if layer == 0 else Cw["nb2_1"]
                for g0 in range(0, npc, 512):
                    gw = min(512, npc - g0)
                    nt = sp.tile([P, 512], F32, tag="nt")
                    nc.sync.dma_start(nt[:, :gw], src_dram[:, g0:g0 + gw])
                    hp = pp.tile([P, 512], F32, tag="z")
                    nc.tensor.matmul(out=hp[:, :gw], lhsT=nw1a[:],
                                     rhs=nt[:, :gw], start=True, stop=False,
                                     skip_group_check=True)
                    nc.tensor.matmul(out=hp[:, :gw], lhsT=nw1b[:],
                                     rhs=agg[:, g0:g0 + gw], start=False,
                                     stop=True, skip_group_check=True)
                    hn = sp.tile([P, 512], F32, tag="hn")
                    nc.scalar.activation(out=hn[:, :gw], in_=hp[:, :gw],
                                         func=AF.Relu, bias=nb1[:, :1])
                    op_ = pp.tile([P, 512], F32, tag="m")
                    nc.tensor.matmul(out=op_[:, :gw], lhsT=nw2[:], rhs=hn[:, :gw],
                                     start=True, stop=True)
                    o_t = sp.tile([P, 512], F32, tag="ot")
                    nc.scalar.activation(out=o_t[:, :gw], in_=op_[:, :gw],
                                         func=AF.Identity, bias=nb2[:, :1])
                    if layer == 0:
                        nc.sync.dma_start(d_n1T[:, g0:g0 + gw], o_t[:, :gw])
                        # P1 shard rows + feed AllGather later
                        for k in range(0, gw, P):
                            pps = pp.tile([P, P], F32, tag="agg")
                            nc.tensor.matmul(pps[:], lhsT=o_t[:, k:k + P],
                                             rhs=W["w1a_1"][:], start=True,
                                             stop=True)
                            stg = sp.tile([P, P], F32, tag="p1stage")
                            nc.vector.tensor_copy(stg[:], pps[:])
                            nc.sync.dma_start(d_p1b[g0 + k:g0 + k + P, :], stg[:])
                    else:
                        fp = pp.tile([P, 512], F32, tag="z")
                        nc.tensor.matmul(out=fp[:, :gw], lhsT=W["fw1"][:],
                                         rhs=o_t[:, :gw], start=True, stop=True)
                        fh = sp.tile([P, 512], F32, tag="fh")
                        nc.scalar.activation(out=fh[:, :gw], in_=fp[:, :gw],
                                             func=AF.Relu, bias=Cw["fb1"][:, :1])
                        fo = pp.tile([P, 512], F32, tag="m")
                        nc.tensor.matmul(out=fo[:, :gw], lhsT=W["fw2"][:],
                                         rhs=fh[:, :gw], start=True, stop=True)
                        fs = sp.tile([P, 512], F32, tag="fs")
                        nc.scalar.activation(out=fs[:, :gw], in_=fo[:, :gw],
                                             func=AF.Identity, bias=Cw["fb2"][:, :1])
                        nc.sync.dma_start(d_outT[:, g0:g0 + gw], fs[:, :gw])

            do = lambda *names: variant in names
            agg0 = edge_phase(0) if do("full", "l0", "l0node", "noag") else None
            if do("full", "l0node", "noag"):
                node_phase(0, agg0)
            if debug:
                st = sp.tile([P, 512], F32, tag="dbgst")
                for g0 in range(0, e_slots, 512):
                    gw = min(512, e_slots - g0)
                    nc.sync.dma_start(st[:, :gw], d_H0T[:, g0:g0 + gw])
                    nc.sync.dma_start(d_dbg["dbg_h0"][:, g0:g0 + gw], st[:, :gw])
                for g0 in range(0, npc, 512):
                    gw = min(512, npc - g0)
                    nc.sync.dma_start(st[:, :gw], d_n1T[:, g0:g0 + gw])
                    nc.sync.dma_start(d_dbg["dbg_n1"][:, g0:g0 + gw], st[:, :gw])
            if do("full"):
                nc.gpsimd.collective_compute(
                    "AllGather", OP.bypass,
                    replica_groups=[list(range(cfg.cores))],
                    ins=[d_p1b[:].opt()], outs=[d_p1f[:].opt()])
            if debug:
                st2 = sp.tile([P, 512], F32, tag="dbgst2")
                for g0 in range(0, cfg.n_pad, 512):
                    gw = min(512, cfg.n_pad - g0)
                    nc.sync.dma_start(st2[:, :gw],
                                      d_p1f[g0:g0 + gw, :].rearrange("(a p) f -> p (a f)", p=P)
                                      if False else d_p1f[g0:g0 + gw, :].rearrange("n f -> f n"))
                    nc.sync.dma_start(d_dbg["dbg_p1f"][g0:g0 + gw, :].rearrange("n f -> f n"), st2[:, :gw])
            if do("full", "noag"):
                agg1 = edge_phase(1)
                node_phase(1, agg1)
            if variant == "floor":
                t0 = sp.tile([P, P], F32, tag="fl")
                nc.sync.dma_start(t0[:], d_n0T[:, :P])
                nc.sync.dma_start(d_outT[:, :P], t0[:])

    nc.compile()
    return nc


# ----------------------------------------------------------------------
# top level
# ----------------------------------------------------------------------

def _assemble(cfg, meta, results):
    n_pad, npc = cfg.n_pad, cfg.npc
    nodesT = np.concatenate([results[c]["out_nodesT"] for c in range(cfg.cores)],
                            axis=1)
    nodes_out = np.ascontiguousarray(nodesT.T[:cfg.n]).astype(np.float32)
    edge_out = np.empty((cfg.e, P), np.float32)
    for c in range(cfg.cores):
        s2e = meta["slot2edge"][c]
        m = s2e >= 0
        edge_out[s2e[m]] = results[c]["edge_out"][m]
    return nodes_out, edge_out


def run_cfg(cfg, inputs, runner=None):
    in_maps, meta = host_prepare(cfg, inputs)
    nc = build_nc(cfg, meta["k_run"])
    res = run_bass_kernel_spmd(nc, in_maps, core_ids=list(range(cfg.cores)))
    return _assemble(cfg, meta, res.results)


def kernel(**inputs):
    return run_cfg(CFG_FULL, inputs)
